# revision 11
# baseline (speedup 1.0000x reference)
"""Trainium2 Bass kernel for the GeneGroupModel two-layer problem.

Model: g = relu(segment_sum(x * w_flat, seg) + gene_b)
       h1 = relu(BN(g @ W1.T + b1));  h2 = relu(BN(h1 @ W2.T + b2))
       out = h2 @ Wout.T + bout            (BN uses full-batch statistics)

Strategy (8 NeuronCores, data-parallel over the batch):
 - batch B=2048 sharded 8 x 256 rows.
 - w_flat is folded into x on the host (xw = x * w_flat, one bf16
   rounding), transposed into a [128, 469*256] streaming layout
   (partition p, column c*256+b = xw[b, 128c+p]) so the device streams
   it with large fully-contiguous DMAs at HBM line rate.
 - The segment structure repeats every 1920 features == 64 groups, so
   the segment-sum is a band matmul against ONE constant 0/1 indicator
   block IND [128, 15*64]: per 128-feature chunk,
   psg[64, 256] += IND_s.T @ xwT.
 - x streams on BOTH HWDGE rings (even super-chunk pairs on sync, odd
   pairs on scalar) so each ring's per-DMA completion bubble hides
   behind the other ring's transfer; the final four chunks are
   single-chunk DMAs so the post-stream matmul tail is short.
 - MLP layer-1 accumulates into 4 persistent PSUM banks interleaved
   with the stream.
 - BN batch statistics: bn_stats on DVE (count/mean/M2 per PSUM bank),
   converted to additive sum/sqsum, AllReduced across the 8 cores.
   Two warm-up AllReduces (gated on mid-stream x tiles, triggered via
   the idle gpsimd queue) pay the ~12us ncfw wake + first-op cost
   before the real AllReduce arrives, so BN1's AllReduce runs hot.
 - BN apply is folded:  relu(scl*x + shf) = scl * relu(x + c) with
   c = (beta/gamma)*std - mu;  the scl factor is folded into the NEXT
   layer's weights on-chip (W2 and Wout column scaling), so the apply
   itself is a cheap DVE add+relu into bf16.
 - b1/b2 are omitted: BN subtracts the batch mean, so a constant bias
   added before BN cancels exactly.
"""

import numpy as np
import ml_dtypes

import concourse.bass as bass
import concourse.bacc as bacc
import concourse.mybir as mybir
from concourse import tile
from concourse.bass_utils import run_bass_kernel_spmd

F32 = mybir.dt.float32
BF16 = mybir.dt.bfloat16

B, F, G = 2048, 60000, 2000
H1, H2 = 512, 256
EPS = 1e-5
NCORES = 8
BS = B // NCORES            # 256 batch rows per core
NSUB = 469                  # ceil(F/128); F padded to FP
FP = NSUB * 128             # 60032
SUPER_SUBS = 15             # 15 x 128 = 1920 features per super-chunk
NSUPER = 32                 # 31 full + 1 tail (4 subchunks, 16 groups)
GBLK = 64                   # groups per full super-chunk
GT_TILES = 16               # partition tiles of gT (G padded to 2048)

_SIZES = np.tile(np.array([16, 24, 32, 48], np.int64), 500)


def _build_graph():
    nc = bacc.Bacc("TRN2", target_bir_lowering=False, debug=False,
                   num_devices=NCORES)
    x_d = nc.declare_dram_parameter("x", [128, NSUB * BS], BF16, isOutput=False)
    ind_d = nc.declare_dram_parameter("ind", [128, SUPER_SUBS * GBLK], BF16, isOutput=False)
    gbpt_d = nc.declare_dram_parameter("gbpt", [128, GT_TILES], F32, isOutput=False)
    w1t_d = nc.declare_dram_parameter("w1t", [128, GT_TILES * H1], BF16, isOutput=False)
    w2t_d = nc.declare_dram_parameter("w2t", [128, 4 * H2], BF16, isOutput=False)
    # packed small consts: bg1 0:4 | gamma1 4:8 | bg2 8:10 | gamma2 10:12
    #                      | wout 12:14 | bout at [0,14]   (bg = beta/gamma)
    bnc_d = nc.declare_dram_parameter("bnc", [128, 16], F32, isOutput=False)
    out_d = nc.declare_dram_parameter("out", [1, BS], F32, isOutput=True)

    AT = mybir.AluOpType
    AF = mybir.ActivationFunctionType

    with tile.TileContext(nc) as tc:
        with (
            tc.tile_pool(name="const", bufs=1) as constp,
            tc.tile_pool(name="xt", bufs=5) as xtp,
            tc.tile_pool(name="gt", bufs=1) as gtp,
            tc.tile_pool(name="mlp", bufs=1) as mlpp,
            tc.tile_pool(name="small", bufs=1) as smallp,
            tc.tile_pool(name="psg", bufs=2, space="PSUM") as psgp,
            tc.tile_pool(name="psh1", bufs=1, space="PSUM") as psh1p,
            tc.tile_pool(name="psh2", bufs=2, space="PSUM") as psh2p,
            tc.tile_pool(name="dram", bufs=1, space="DRAM") as dramp,
        ):
            # ---------------- x stream + constants ----------------
            # first pair of super-chunks on the sync ring immediately
            xt0 = xtp.tile([128, 2 * SUPER_SUBS * BS], BF16, tag="xt")
            nc.sync.dma_start(xt0[:], x_d[:, 0:2 * SUPER_SUBS * BS])

            # constants split across the two rings to balance bytes
            w1t = constp.tile([128, GT_TILES * H1], BF16)
            nc.sync.dma_start(w1t[:, :8 * H1], w1t_d[:, :8 * H1])
            ind_sb = constp.tile([128, SUPER_SUBS * GBLK], BF16)
            nc.scalar.dma_start(ind_sb[:], ind_d[:])
            gbpt = constp.tile([128, GT_TILES], F32)
            nc.scalar.dma_start(gbpt[:], gbpt_d[:])
            nc.scalar.dma_start(w1t[:, 8 * H1:], w1t_d[:, 8 * H1:])
            w2t = constp.tile([128, 4 * H2], BF16)
            nc.scalar.dma_start(w2t[:], w2t_d[:])
            bnc = constp.tile([128, 16], F32)
            nc.scalar.dma_start(bnc[:], bnc_d[:])

            epst = constp.tile([128, 1], F32)
            nc.vector.memset(epst[:], EPS)
            # preload the ACT Sqrt table off the critical path
            sqdummy = constp.tile([128, 1], F32)
            nc.scalar.activation(sqdummy[:], epst[:], AF.Sqrt, bias=epst[:])

            # gT accumulator [2048(G padded) x 256] bf16: 16 partition-tiles
            # side by side.  Groups 2000..2047 are never written -> zero.
            gt = gtp.tile([128, GT_TILES * BS], BF16)
            nc.vector.memset(gt[64:128, 15 * BS:16 * BS], 0.0)

            # layer-1 pre-activations accumulate here across the main loop
            h1p = psh1p.tile([128, 4 * 512], F32)   # 4 PSUM banks, cols 0:BS used

            # warm-up collective plumbing (sources gated on mid-stream tiles)
            warm_srcA = smallp.tile([128, 1], F32)
            warm_srcB = smallp.tile([128, 1], F32)
            warmA_in = dramp.tile([128, 1], F32)
            warmA_out = dramp.tile([128, 1], F32)
            warmB_in = dramp.tile([128, 1], F32)
            warmB_out = dramp.tile([128, 1], F32)

            def l1_matmul(k):
                # h1p[:, m] += W1T[k-block].T @ gt_k   (4 banks, 16-step accum)
                for m in range(4):
                    nc.tensor.matmul(
                        h1p[:, m * 512:m * 512 + BS],
                        w1t[:, k * H1 + m * 128:k * H1 + (m + 1) * 128],
                        gt[:, k * BS:(k + 1) * BS],
                        start=(k == 0), stop=(k == GT_TILES - 1))

            # final four chunks land as separate single-chunk DMAs
            tails = [constp.tile([128, SUPER_SUBS * BS], BF16, name=f"xtl{i}")
                     for i in range(3)]
            tails.append(constp.tile([128, 4 * BS], BF16, name="xtl3"))

            # ---------------- segment-sum main loop ----------------
            xts = {0: xt0}
            for t in range(NSUPER):
                nsub = SUPER_SUBS if t < NSUPER - 1 else 4
                ng = GBLK if t < NSUPER - 1 else 16
                if t % 2 == 0 and 2 <= t <= 26:
                    p = t // 2
                    ring = nc.sync if p % 2 == 0 else nc.scalar
                    xt = xtp.tile([128, 2 * SUPER_SUBS * BS], BF16, tag="xt")
                    c0 = 2 * p * SUPER_SUBS * BS
                    ring.dma_start(xt[:], x_d[:, c0:c0 + 2 * SUPER_SUBS * BS])
                    xts[2 * p] = xt
                    if t == 26:
                        nc.sync.dma_start(tails[0][:], x_d[:, 420 * BS:435 * BS])
                        nc.scalar.dma_start(tails[1][:], x_d[:, 435 * BS:450 * BS])
                        nc.sync.dma_start(tails[2][:], x_d[:, 450 * BS:465 * BS])
                        nc.sync.dma_start(tails[3][:], x_d[:, 465 * BS:469 * BS])
                if t < 28:
                    xt = xts[t - (t % 2)]
                    off = (t % 2) * SUPER_SUBS
                else:
                    xt, off = tails[t - 28], 0
                psg = psgp.tile([64, 512], F32, tag="psg")
                for s in range(nsub):
                    nc.tensor.matmul(psg[:, 0:BS], ind_sb[:, s * GBLK:(s + 1) * GBLK],
                                     xt[:, (off + s) * BS:(off + s + 1) * BS],
                                     start=(s == 0), stop=(s == nsub - 1))
                # gt[64t : 64t+ng, :] = relu(psg + gene_b)
                j, po = t // 2, 64 * (t % 2)
                nc.vector.tensor_scalar(
                    gt[po:po + ng, j * BS:(j + 1) * BS],
                    psg[0:ng, 0:BS],
                    gbpt[po:po + ng, j:j + 1],
                    0.0,
                    op0=AT.add,
                    op1=AT.max,
                )
                # interleave layer-1 accumulation
                if t % 2 == 1:
                    if 5 <= t <= 29:
                        l1_matmul((t - 5) // 2)
                        if t == 29:
                            l1_matmul(13)
                    elif t == 31:
                        l1_matmul(14)
                # warm-up collectives: A pays the ncfw wake early; B lands
                # right before the BN1 AllReduce so that one runs hot.
                # Triggered via the otherwise-idle gpsimd queue.
                if t == 16:
                    nc.vector.tensor_copy(warm_srcA[:], xts[16][0:128, 0:1])
                    nc.gpsimd.dma_start(warmA_in[:], warm_srcA[:])
                    nc.gpsimd.collective_compute(
                        "AllReduce", AT.add,
                        replica_groups=[list(range(NCORES))],
                        ins=[warmA_in.opt()], outs=[warmA_out.opt()])
                elif t == 26:
                    nc.vector.tensor_copy(warm_srcB[:], xts[26][0:128, 0:1])
                    nc.gpsimd.dma_start(warmB_in[:], warm_srcB[:])
                    nc.gpsimd.collective_compute(
                        "AllReduce", AT.add,
                        replica_groups=[list(range(NCORES))],
                        ins=[warmB_in.opt()], outs=[warmB_out.opt()])
            l1_matmul(15)

            # ---------------- BN1 stats (all-DVE bn_stats) ----------------
            # bn_stats gives (count, mean, count*var) for even/odd halves;
            # convert to additive (sum, sqsum) for the AllReduce.
            stats1 = smallp.tile([128, 8], F32)    # sums 0:4 | sqsums 4:8
            bnout1 = smallp.tile([128, 24], F32)
            for m in range(4):
                nc.vector.bn_stats(bnout1[:, 6 * m:6 * m + 6],
                                   h1p[:, m * 512:m * 512 + BS])
            r1 = bnout1[:].rearrange("p (g s) -> p s g", g=4, s=6)

            def srow(r, s):
                return r[:, s:s + 1, :].squeeze(1)

            def cvt_stats(r, stats, nb, tmp):
                ta, tb, tc = tmp
                nc.vector.tensor_tensor(ta[:], srow(r, 1), srow(r, 4), op=AT.add)
                nc.vector.tensor_scalar_mul(stats[:, 0:nb], ta[:], float(BS // 2))
                nc.vector.tensor_tensor(tb[:], srow(r, 1), srow(r, 1), op=AT.mult)
                nc.vector.tensor_tensor(tc[:], srow(r, 4), srow(r, 4), op=AT.mult)
                nc.vector.tensor_tensor(tb[:], tb[:], tc[:], op=AT.add)
                nc.vector.tensor_scalar_mul(tb[:], tb[:], float(BS // 2))
                nc.vector.tensor_tensor(tc[:], srow(r, 2), srow(r, 5), op=AT.add)
                nc.vector.tensor_tensor(stats[:, nb:2 * nb], tb[:], tc[:], op=AT.add)

            tmp1a = smallp.tile([128, 4], F32)
            tmp1b = smallp.tile([128, 4], F32)
            tmp1c = smallp.tile([128, 4], F32)
            cvt_stats(r1, stats1, 4, (tmp1a, tmp1b, tmp1c))

            totals1 = smallp.tile([128, 8], F32)
            bn1_in = dramp.tile([128, 8], F32)
            bn1_out = dramp.tile([128, 8], F32)
            nc.sync.dma_start(bn1_in[:], stats1[:])
            nc.gpsimd.collective_compute(
                "AllReduce", AT.add,
                replica_groups=[list(range(NCORES))],
                ins=[bn1_in.opt()], outs=[bn1_out.opt()])
            nc.sync.dma_start(totals1[:], bn1_out[:])

            # ---------------- BN1 math ----------------
            #   scl = gamma * rsqrt(var+eps);  h1 = scl * relu(h1p + c)
            #   with c = (beta/gamma)*std - mu; scl folded into W2 columns.
            mub1 = smallp.tile([128, 8], F32)
            nc.vector.tensor_scalar_mul(mub1[:], totals1[:], 1.0 / B)
            var1 = smallp.tile([128, 4], F32)
            nc.vector.tensor_tensor(var1[:], mub1[:, 0:4], mub1[:, 0:4],
                                    op=AT.mult)
            nc.vector.tensor_tensor(var1[:], mub1[:, 4:8], var1[:],
                                    op=AT.subtract)
            std1 = smallp.tile([128, 4], F32)
            nc.scalar.activation(std1[:], var1[:], AF.Sqrt, bias=epst[:])
            rstd1 = smallp.tile([128, 4], F32)
            nc.vector.reciprocal(rstd1[:], std1[:])
            scl1 = smallp.tile([128, 4], F32)
            nc.vector.tensor_tensor(scl1[:], bnc[:, 4:8], rstd1[:], op=AT.mult)
            c1 = smallp.tile([128, 4], F32)
            nc.vector.tensor_tensor(c1[:], bnc[:, 0:4], std1[:], op=AT.mult)
            nc.vector.tensor_tensor(c1[:], c1[:], mub1[:, 0:4], op=AT.subtract)

            # apply + scaled W2, then layer-2 matmuls per k-block
            h1 = mlpp.tile([128, 4 * BS], BF16)
            w2s = mlpp.tile([128, 4 * H2], BF16)
            ph2s = []
            for m in range(2):
                ph2 = psh2p.tile([128, 512], F32, tag="ph2")
                ph2s.append(ph2)
            for k in range(4):
                nc.scalar.activation(w2s[:, k * H2:(k + 1) * H2],
                                     w2t[:, k * H2:(k + 1) * H2],
                                     AF.Copy, scale=scl1[:, k:k + 1])
                nc.vector.tensor_scalar(
                    h1[:, k * BS:(k + 1) * BS],
                    h1p[:, k * 512:k * 512 + BS],
                    c1[:, k:k + 1], 0.0, op0=AT.add, op1=AT.max)
                for m in range(2):
                    nc.tensor.matmul(
                        ph2s[m][:, 0:BS],
                        w2s[:, k * H2 + m * 128:k * H2 + (m + 1) * 128],
                        h1[:, k * BS:(k + 1) * BS],
                        start=(k == 0), stop=(k == 3))

            # ---------------- BN2 stats + AllReduce ----------------
            stats2 = smallp.tile([128, 4], F32)    # sums 0:2 | sqsums 2:4
            bnout2 = smallp.tile([128, 12], F32)
            for m in range(2):
                nc.vector.bn_stats(bnout2[:, 6 * m:6 * m + 6],
                                   ph2s[m][:, 0:BS])
            r2 = bnout2[:].rearrange("p (g s) -> p s g", g=2, s=6)
            tmp2a = smallp.tile([128, 2], F32)
            tmp2b = smallp.tile([128, 2], F32)
            tmp2c = smallp.tile([128, 2], F32)
            cvt_stats(r2, stats2, 2, (tmp2a, tmp2b, tmp2c))

            totals2 = smallp.tile([128, 4], F32)
            bn2_in = dramp.tile([128, 4], F32)
            bn2_out = dramp.tile([128, 4], F32)
            nc.sync.dma_start(bn2_in[:], stats2[:])
            nc.gpsimd.collective_compute(
                "AllReduce", AT.add,
                replica_groups=[list(range(NCORES))],
                ins=[bn2_in.opt()], outs=[bn2_out.opt()])
            nc.sync.dma_start(totals2[:], bn2_out[:])

            # ---------------- BN2 math + head ----------------
            mub2 = smallp.tile([128, 4], F32)
            nc.vector.tensor_scalar_mul(mub2[:], totals2[:], 1.0 / B)
            var2 = smallp.tile([128, 2], F32)
            nc.vector.tensor_tensor(var2[:], mub2[:, 0:2], mub2[:, 0:2],
                                    op=AT.mult)
            nc.vector.tensor_tensor(var2[:], mub2[:, 2:4], var2[:],
                                    op=AT.subtract)
            std2 = smallp.tile([128, 2], F32)
            nc.scalar.activation(std2[:], var2[:], AF.Sqrt, bias=epst[:])
            rstd2 = smallp.tile([128, 2], F32)
            nc.vector.reciprocal(rstd2[:], std2[:])
            scl2 = smallp.tile([128, 2], F32)
            nc.vector.tensor_tensor(scl2[:], bnc[:, 10:12], rstd2[:], op=AT.mult)
            c2 = smallp.tile([128, 2], F32)
            nc.vector.tensor_tensor(c2[:], bnc[:, 8:10], std2[:], op=AT.mult)
            nc.vector.tensor_tensor(c2[:], c2[:], mub2[:, 0:2], op=AT.subtract)
            wouts = smallp.tile([128, 2], BF16)
            nc.vector.tensor_tensor(wouts[:], bnc[:, 12:14], scl2[:], op=AT.mult)

            h2 = mlpp.tile([128, 2 * BS], BF16)
            pso = psh2p.tile([128, 512], F32, tag="ph2")
            for k in range(2):
                nc.vector.tensor_scalar(
                    h2[:, k * BS:(k + 1) * BS], ph2s[k][:, 0:BS],
                    c2[:, k:k + 1], 0.0, op0=AT.add, op1=AT.max)
                nc.tensor.matmul(pso[0:1, 0:BS], wouts[:, k:k + 1],
                                 h2[:, k * BS:(k + 1) * BS],
                                 start=(k == 0), stop=(k == 1))
            outsb = smallp.tile([1, BS], F32)
            nc.scalar.activation(outsb[:], pso[0:1, 0:BS], AF.Identity,
                                 bias=bnc[0:1, 14:15])
            nc.sync.dma_start(out_d[:], outsb[:])

    nc.compile()
    return nc


def _pack_pt(v, ncols):
    """[N] -> [128, ncols] with element (p, c) = v[128c + p], zero padded."""
    full = np.zeros(128 * ncols, np.float32)
    full[:v.shape[0]] = v
    return np.ascontiguousarray(full.reshape(ncols, 128).T)


_GRAPH = None


def _prepare_in_maps(x, seg, w_flat, gene_b, W1, b1, gamma1, beta1, W2, b2,
                     gamma2, beta2, Wout, bout):
    x = np.asarray(x, np.float32)
    seg = np.asarray(seg)
    exp_seg = np.repeat(np.arange(G, dtype=np.int64), _SIZES)
    assert np.array_equal(seg.astype(np.int64), exp_seg), "unexpected seg layout"

    # fold w into x (single bf16 rounding), pad to FP, per-core transpose
    # to the [128, NSUB*BS] streaming layout: xr[p, c*BS+b] = xw[b, 128c+p]
    xw = x * np.asarray(w_flat, np.float32)[None, :]
    xb = np.zeros((B, FP), ml_dtypes.bfloat16)
    xb[:, :F] = xw.astype(ml_dtypes.bfloat16)
    xr = np.ascontiguousarray(
        xb.view(np.uint16).reshape(NCORES, BS, NSUB, 128).transpose(0, 3, 2, 1)
    ).reshape(NCORES, 128, NSUB * BS).view(ml_dtypes.bfloat16)

    ind = (exp_seg[:SUPER_SUBS * 128, None] == np.arange(GBLK)[None, :])
    ind = np.ascontiguousarray(
        ind.astype(ml_dtypes.bfloat16).reshape(SUPER_SUBS, 128, GBLK)
        .transpose(1, 0, 2).reshape(128, SUPER_SUBS * GBLK))
    gbpt = _pack_pt(np.asarray(gene_b, np.float32), GT_TILES)
    w1t_full = np.zeros((GT_TILES * 128, H1), np.float32)
    w1t_full[:G] = np.asarray(W1, np.float32).T
    w1t = np.ascontiguousarray(
        w1t_full.reshape(GT_TILES, 128, H1).transpose(1, 0, 2)
        .reshape(128, GT_TILES * H1)).astype(ml_dtypes.bfloat16)
    w2t = np.ascontiguousarray(
        np.asarray(W2, np.float32).T.reshape(4, 128, H2).transpose(1, 0, 2)
        .reshape(128, 4 * H2)).astype(ml_dtypes.bfloat16)

    ga1 = np.asarray(gamma1, np.float32)
    be1 = np.asarray(beta1, np.float32)
    ga2 = np.asarray(gamma2, np.float32)
    be2 = np.asarray(beta2, np.float32)
    bg1 = np.divide(be1, ga1, out=np.zeros_like(be1), where=(ga1 != 0))
    bg2 = np.divide(be2, ga2, out=np.zeros_like(be2), where=(ga2 != 0))

    bnc = np.zeros((128, 16), np.float32)
    bnc[:, 0:4] = _pack_pt(bg1, 4)
    bnc[:, 4:8] = _pack_pt(ga1, 4)
    bnc[:, 8:10] = _pack_pt(bg2, 2)
    bnc[:, 10:12] = _pack_pt(ga2, 2)
    bnc[:, 12:14] = _pack_pt(np.asarray(Wout, np.float32).reshape(-1), 2)
    bnc[0, 14] = np.asarray(bout, np.float32).reshape(-1)[0]

    consts = dict(ind=ind, gbpt=gbpt, w1t=w1t, w2t=w2t, bnc=bnc)
    return [dict(consts, x=np.ascontiguousarray(xr[i]))
            for i in range(NCORES)]


def _graph():
    global _GRAPH
    if _GRAPH is None:
        _GRAPH = _build_graph()
    return _GRAPH


def _gather(res):
    out = np.concatenate([np.asarray(r["out"]).reshape(-1)
                          for r in res.results])
    return out.reshape(B, 1).astype(np.float32)


def kernel(**inputs):
    in_maps = _prepare_in_maps(**inputs)
    res = run_bass_kernel_spmd(_graph(), in_maps, list(range(NCORES)))
    return _gather(res)


# revision 15
# speedup vs baseline: 1.2035x; 1.2035x over previous
"""Trainium2 Bass kernel for the GeneGroupModel two-layer problem.

Model: g = relu(segment_sum(x * w_flat, seg) + gene_b)
       h1 = relu(BN(g @ W1.T + b1));  h2 = relu(BN(h1 @ W2.T + b2))
       out = h2 @ Wout.T + bout            (BN uses full-batch statistics)

Strategy (8 NeuronCores, data-parallel over the batch):
 - batch B=2048 sharded 8 x 256 rows.
 - w_flat is folded into x on the host (xw = x * w_flat, one bf16
   rounding), transposed into a [128, 469*256] streaming layout
   (partition p, column c*256+b = xw[b, 128c+p]) so the device streams
   it with large fully-contiguous DMAs at HBM line rate.
 - The segment structure repeats every 1920 features == 64 groups, so
   the segment-sum is a band matmul against ONE constant 0/1 indicator
   block IND [128, 15*64]: per 128-feature chunk,
   psg[64, 256] += IND_s.T @ xwT.
 - x streams on BOTH HWDGE rings (even super-chunk pairs on sync, odd
   pairs on scalar) so each ring's per-DMA completion bubble hides
   behind the other ring's transfer; the final four chunks are
   single-chunk DMAs so the post-stream matmul tail is short.
 - MLP layer-1 accumulates into 4 persistent PSUM banks interleaved
   with the stream.
 - BN batch statistics: bn_stats on DVE (count/mean/M2 per PSUM bank),
   converted to additive sum/sqsum, AllReduced across the 8 cores.
   Two warm-up AllReduces (gated on mid-stream x tiles, triggered via
   the idle gpsimd queue) pay the ~12us ncfw wake + first-op cost
   before the real AllReduce arrives, so BN1's AllReduce runs hot.
 - BN apply is folded:  relu(scl*x + shf) = scl * relu(x + c) with
   c = (beta/gamma)*std - mu;  the scl factor is folded into the NEXT
   layer's weights on-chip (W2 and Wout column scaling), so the apply
   itself is a cheap DVE add+relu into bf16.
 - b1/b2 are omitted: BN subtracts the batch mean, so a constant bias
   added before BN cancels exactly.
"""

import numpy as np
import ml_dtypes

import concourse.bass as bass
import concourse.bacc as bacc
import concourse.mybir as mybir
from concourse import tile
from concourse.bass_utils import run_bass_kernel_spmd

F32 = mybir.dt.float32
BF16 = mybir.dt.bfloat16

B, F, G = 2048, 60000, 2000
H1, H2 = 512, 256
EPS = 1e-5
NCORES = 8
BS = B // NCORES            # 256 batch rows per core
NSUB = 469                  # ceil(F/128); F padded to FP
FP = NSUB * 128             # 60032
SUPER_SUBS = 15             # 15 x 128 = 1920 features per super-chunk
NSUPER = 32                 # 31 full + 1 tail (4 subchunks, 16 groups)
GBLK = 64                   # groups per full super-chunk
GT_TILES = 16               # partition tiles of gT (G padded to 2048)

_SIZES = np.tile(np.array([16, 24, 32, 48], np.int64), 500)


def _build_graph():
    nc = bacc.Bacc("TRN2", target_bir_lowering=False, debug=False,
                   num_devices=NCORES)
    x_d = nc.declare_dram_parameter("x", [128, NSUB * BS], BF16, isOutput=False)
    ind_d = nc.declare_dram_parameter("ind", [128, SUPER_SUBS * GBLK], BF16, isOutput=False)
    gbpt_d = nc.declare_dram_parameter("gbpt", [128, GT_TILES], F32, isOutput=False)
    w1t_d = nc.declare_dram_parameter("w1t", [128, GT_TILES * H1], BF16, isOutput=False)
    w2t_d = nc.declare_dram_parameter("w2t", [128, 4 * H2], BF16, isOutput=False)
    # packed small consts: bg1 0:4 | gamma1 4:8 | bg2 8:10 | gamma2 10:12
    #                      | wout 12:14 | bout at [0,14]   (bg = beta/gamma)
    bnc_d = nc.declare_dram_parameter("bnc", [128, 16], F32, isOutput=False)
    out_d = nc.declare_dram_parameter("out", [1, BS], F32, isOutput=True)

    AT = mybir.AluOpType
    AF = mybir.ActivationFunctionType

    with tile.TileContext(nc) as tc:
        with (
            tc.tile_pool(name="const", bufs=1) as constp,
            tc.tile_pool(name="xt", bufs=5) as xtp,
            tc.tile_pool(name="gt", bufs=1) as gtp,
            tc.tile_pool(name="mlp", bufs=1) as mlpp,
            tc.tile_pool(name="small", bufs=1) as smallp,
            tc.tile_pool(name="psg", bufs=2, space="PSUM") as psgp,
            tc.tile_pool(name="psh1", bufs=1, space="PSUM") as psh1p,
            tc.tile_pool(name="psh2", bufs=2, space="PSUM") as psh2p,
            tc.tile_pool(name="dram", bufs=1, space="DRAM") as dramp,
        ):
            # ---------------- x stream + constants ----------------
            # x streams exclusively on the sync ring; first pair immediately
            xt0 = xtp.tile([128, 2 * SUPER_SUBS * BS], BF16, tag="xt")
            nc.sync.dma_start(xt0[:], x_d[:, 0:2 * SUPER_SUBS * BS])

            # all constants on the scalar ring (never displace x)
            ind_sb = constp.tile([128, SUPER_SUBS * GBLK], BF16)
            nc.scalar.dma_start(ind_sb[:], ind_d[:])
            gbpt = constp.tile([128, GT_TILES], F32)
            nc.scalar.dma_start(gbpt[:], gbpt_d[:])
            w1t = constp.tile([128, GT_TILES * H1], BF16)
            nc.scalar.dma_start(w1t[:], w1t_d[:])
            w2t = constp.tile([128, 4 * H2], BF16)
            nc.scalar.dma_start(w2t[:], w2t_d[:])
            bnc = constp.tile([128, 16], F32)
            nc.scalar.dma_start(bnc[:], bnc_d[:])

            epst = constp.tile([128, 1], F32)
            nc.vector.memset(epst[:], EPS)
            # preload the ACT Sqrt table off the critical path
            sqdummy = constp.tile([128, 1], F32)
            nc.scalar.activation(sqdummy[:], epst[:], AF.Sqrt, bias=epst[:])

            # gT accumulator [2048(G padded) x 256] bf16: 16 partition-tiles
            # side by side.  Groups 2000..2047 are never written -> zero.
            gt = gtp.tile([128, GT_TILES * BS], BF16)
            nc.vector.memset(gt[64:128, 15 * BS:16 * BS], 0.0)

            # layer-1 pre-activations accumulate here across the main loop
            h1p = psh1p.tile([128, 4 * 512], F32)   # 4 PSUM banks, cols 0:BS used

            # warm-up collective plumbing (source gated on a mid-stream tile)
            warm_src = smallp.tile([128, 1], F32)
            warm_in = dramp.tile([128, 1], F32)
            warm_out = dramp.tile([128, 1], F32)

            def l1_matmul(k):
                # h1p[:, m] += W1T[k-block].T @ gt_k   (4 banks, 16-step accum)
                for m in range(4):
                    nc.tensor.matmul(
                        h1p[:, m * 512:m * 512 + BS],
                        w1t[:, k * H1 + m * 128:k * H1 + (m + 1) * 128],
                        gt[:, k * BS:(k + 1) * BS],
                        start=(k == 0), stop=(k == GT_TILES - 1))

            # final four chunks land as separate single-chunk DMAs
            tails = [constp.tile([128, SUPER_SUBS * BS], BF16, name=f"xtl{i}")
                     for i in range(3)]
            tails.append(constp.tile([128, 4 * BS], BF16, name="xtl3"))

            # ---------------- segment-sum main loop ----------------
            xts = {0: xt0}
            for t in range(NSUPER):
                nsub = SUPER_SUBS if t < NSUPER - 1 else 4
                ng = GBLK if t < NSUPER - 1 else 16
                if t % 2 == 0 and 2 <= t <= 26:
                    p = t // 2
                    xt = xtp.tile([128, 2 * SUPER_SUBS * BS], BF16, tag="xt")
                    c0 = 2 * p * SUPER_SUBS * BS
                    nc.sync.dma_start(xt[:], x_d[:, c0:c0 + 2 * SUPER_SUBS * BS])
                    xts[2 * p] = xt
                    if t == 26:
                        nc.sync.dma_start(tails[0][:], x_d[:, 420 * BS:435 * BS])
                        nc.sync.dma_start(tails[1][:], x_d[:, 435 * BS:450 * BS])
                        nc.sync.dma_start(tails[2][:], x_d[:, 450 * BS:465 * BS])
                        nc.sync.dma_start(tails[3][:], x_d[:, 465 * BS:469 * BS])
                if t < 28:
                    xt = xts[t - (t % 2)]
                    off = (t % 2) * SUPER_SUBS
                else:
                    xt, off = tails[t - 28], 0
                psg = psgp.tile([64, 512], F32, tag="psg")
                for s in range(nsub):
                    nc.tensor.matmul(psg[:, 0:BS], ind_sb[:, s * GBLK:(s + 1) * GBLK],
                                     xt[:, (off + s) * BS:(off + s + 1) * BS],
                                     start=(s == 0), stop=(s == nsub - 1))
                # gt[64t : 64t+ng, :] = relu(psg + gene_b)
                j, po = t // 2, 64 * (t % 2)
                nc.vector.tensor_scalar(
                    gt[po:po + ng, j * BS:(j + 1) * BS],
                    psg[0:ng, 0:BS],
                    gbpt[po:po + ng, j:j + 1],
                    0.0,
                    op0=AT.add,
                    op1=AT.max,
                )
                # interleave layer-1 accumulation
                if t % 2 == 1:
                    if 5 <= t <= 29:
                        l1_matmul((t - 5) // 2)
                        if t == 29:
                            l1_matmul(13)
                    elif t == 31:
                        l1_matmul(14)
                # one warm-up collective, gated on pair-10 x data (~88us):
                # it pays the ~25us ncfw wake + first-op cost and retires just
                # before the BN1 AllReduce's doorbell, which then runs hot.
                if t == 20:
                    nc.vector.tensor_copy(warm_src[:], xts[20][0:128, 0:1])
                    nc.gpsimd.dma_start(warm_in[:], warm_src[:])
                    nc.gpsimd.collective_compute(
                        "AllReduce", AT.add,
                        replica_groups=[list(range(NCORES))],
                        ins=[warm_in.opt()], outs=[warm_out.opt()])
            l1_matmul(15)

            # ---------------- BN1 stats (all-DVE bn_stats) ----------------
            # bn_stats gives (count, mean, count*var) for even/odd halves;
            # convert to additive (sum, sqsum) for the AllReduce.
            stats1 = smallp.tile([128, 8], F32)    # sums 0:4 | sqsums 4:8
            bnout1 = smallp.tile([128, 24], F32)
            for m in range(4):
                nc.vector.bn_stats(bnout1[:, 6 * m:6 * m + 6],
                                   h1p[:, m * 512:m * 512 + BS])
            r1 = bnout1[:].rearrange("p (g s) -> p s g", g=4, s=6)

            def srow(r, s):
                return r[:, s:s + 1, :].squeeze(1)

            def cvt_stats(r, stats, nb, tmp):
                ta, tb, tc = tmp
                nc.vector.tensor_tensor(ta[:], srow(r, 1), srow(r, 4), op=AT.add)
                nc.vector.tensor_scalar_mul(stats[:, 0:nb], ta[:], float(BS // 2))
                nc.vector.tensor_tensor(tb[:], srow(r, 1), srow(r, 1), op=AT.mult)
                nc.vector.tensor_tensor(tc[:], srow(r, 4), srow(r, 4), op=AT.mult)
                nc.vector.tensor_tensor(tb[:], tb[:], tc[:], op=AT.add)
                nc.vector.tensor_scalar_mul(tb[:], tb[:], float(BS // 2))
                nc.vector.tensor_tensor(tc[:], srow(r, 2), srow(r, 5), op=AT.add)
                nc.vector.tensor_tensor(stats[:, nb:2 * nb], tb[:], tc[:], op=AT.add)

            tmp1a = smallp.tile([128, 4], F32)
            tmp1b = smallp.tile([128, 4], F32)
            tmp1c = smallp.tile([128, 4], F32)
            cvt_stats(r1, stats1, 4, (tmp1a, tmp1b, tmp1c))

            totals1 = smallp.tile([128, 8], F32)
            bn1_in = dramp.tile([128, 8], F32)
            bn1_out = dramp.tile([128, 8], F32)
            nc.sync.dma_start(bn1_in[:], stats1[:])
            nc.gpsimd.collective_compute(
                "AllReduce", AT.add,
                replica_groups=[list(range(NCORES))],
                ins=[bn1_in.opt()], outs=[bn1_out.opt()])
            nc.sync.dma_start(totals1[:], bn1_out[:])

            # ---------------- BN1 math ----------------
            #   scl = gamma * rsqrt(var+eps);  h1 = scl * relu(h1p + c)
            #   with c = (beta/gamma)*std - mu; scl folded into W2 columns.
            mub1 = smallp.tile([128, 8], F32)
            nc.vector.tensor_scalar_mul(mub1[:], totals1[:], 1.0 / B)
            var1 = smallp.tile([128, 4], F32)
            nc.vector.tensor_tensor(var1[:], mub1[:, 0:4], mub1[:, 0:4],
                                    op=AT.mult)
            nc.vector.tensor_tensor(var1[:], mub1[:, 4:8], var1[:],
                                    op=AT.subtract)
            std1 = smallp.tile([128, 4], F32)
            nc.scalar.activation(std1[:], var1[:], AF.Sqrt, bias=epst[:])
            rstd1 = smallp.tile([128, 4], F32)
            nc.vector.reciprocal(rstd1[:], std1[:])
            scl1 = smallp.tile([128, 4], F32)
            nc.vector.tensor_tensor(scl1[:], bnc[:, 4:8], rstd1[:], op=AT.mult)
            c1 = smallp.tile([128, 4], F32)
            nc.vector.tensor_tensor(c1[:], bnc[:, 0:4], std1[:], op=AT.mult)
            nc.vector.tensor_tensor(c1[:], c1[:], mub1[:, 0:4], op=AT.subtract)

            # apply + scaled W2, then layer-2 matmuls per k-block
            h1 = mlpp.tile([128, 4 * BS], BF16)
            w2s = mlpp.tile([128, 4 * H2], BF16)
            ph2s = []
            for m in range(2):
                ph2 = psh2p.tile([128, 512], F32, tag="ph2")
                ph2s.append(ph2)
            for k in range(4):
                nc.scalar.activation(w2s[:, k * H2:(k + 1) * H2],
                                     w2t[:, k * H2:(k + 1) * H2],
                                     AF.Copy, scale=scl1[:, k:k + 1])
                nc.vector.tensor_scalar(
                    h1[:, k * BS:(k + 1) * BS],
                    h1p[:, k * 512:k * 512 + BS],
                    c1[:, k:k + 1], 0.0, op0=AT.add, op1=AT.max)
                for m in range(2):
                    nc.tensor.matmul(
                        ph2s[m][:, 0:BS],
                        w2s[:, k * H2 + m * 128:k * H2 + (m + 1) * 128],
                        h1[:, k * BS:(k + 1) * BS],
                        start=(k == 0), stop=(k == 3))

            # ---------------- BN2 stats + AllReduce ----------------
            stats2 = smallp.tile([128, 4], F32)    # sums 0:2 | sqsums 2:4
            bnout2 = smallp.tile([128, 12], F32)
            for m in range(2):
                nc.vector.bn_stats(bnout2[:, 6 * m:6 * m + 6],
                                   ph2s[m][:, 0:BS])
            r2 = bnout2[:].rearrange("p (g s) -> p s g", g=2, s=6)
            tmp2a = smallp.tile([128, 2], F32)
            tmp2b = smallp.tile([128, 2], F32)
            tmp2c = smallp.tile([128, 2], F32)
            cvt_stats(r2, stats2, 2, (tmp2a, tmp2b, tmp2c))

            totals2 = smallp.tile([128, 4], F32)
            bn2_in = dramp.tile([128, 4], F32)
            bn2_out = dramp.tile([128, 4], F32)
            nc.sync.dma_start(bn2_in[:], stats2[:])
            nc.gpsimd.collective_compute(
                "AllReduce", AT.add,
                replica_groups=[list(range(NCORES))],
                ins=[bn2_in.opt()], outs=[bn2_out.opt()])
            nc.sync.dma_start(totals2[:], bn2_out[:])

            # ---------------- BN2 math + head ----------------
            mub2 = smallp.tile([128, 4], F32)
            nc.vector.tensor_scalar_mul(mub2[:], totals2[:], 1.0 / B)
            var2 = smallp.tile([128, 2], F32)
            nc.vector.tensor_tensor(var2[:], mub2[:, 0:2], mub2[:, 0:2],
                                    op=AT.mult)
            nc.vector.tensor_tensor(var2[:], mub2[:, 2:4], var2[:],
                                    op=AT.subtract)
            std2 = smallp.tile([128, 2], F32)
            nc.scalar.activation(std2[:], var2[:], AF.Sqrt, bias=epst[:])
            rstd2 = smallp.tile([128, 2], F32)
            nc.vector.reciprocal(rstd2[:], std2[:])
            scl2 = smallp.tile([128, 2], F32)
            nc.vector.tensor_tensor(scl2[:], bnc[:, 10:12], rstd2[:], op=AT.mult)
            c2 = smallp.tile([128, 2], F32)
            nc.vector.tensor_tensor(c2[:], bnc[:, 8:10], std2[:], op=AT.mult)
            nc.vector.tensor_tensor(c2[:], c2[:], mub2[:, 0:2], op=AT.subtract)
            wouts = smallp.tile([128, 2], BF16)
            nc.vector.tensor_tensor(wouts[:], bnc[:, 12:14], scl2[:], op=AT.mult)

            h2 = mlpp.tile([128, 2 * BS], BF16)
            pso = psh2p.tile([128, 512], F32, tag="ph2")
            for k in range(2):
                nc.vector.tensor_scalar(
                    h2[:, k * BS:(k + 1) * BS], ph2s[k][:, 0:BS],
                    c2[:, k:k + 1], 0.0, op0=AT.add, op1=AT.max)
                nc.tensor.matmul(pso[0:1, 0:BS], wouts[:, k:k + 1],
                                 h2[:, k * BS:(k + 1) * BS],
                                 start=(k == 0), stop=(k == 1))
            outsb = smallp.tile([1, BS], F32)
            nc.scalar.activation(outsb[:], pso[0:1, 0:BS], AF.Identity,
                                 bias=bnc[0:1, 14:15])
            nc.sync.dma_start(out_d[:], outsb[:])

    nc.compile()
    return nc


def _pack_pt(v, ncols):
    """[N] -> [128, ncols] with element (p, c) = v[128c + p], zero padded."""
    full = np.zeros(128 * ncols, np.float32)
    full[:v.shape[0]] = v
    return np.ascontiguousarray(full.reshape(ncols, 128).T)


_GRAPH = None


def _prepare_in_maps(x, seg, w_flat, gene_b, W1, b1, gamma1, beta1, W2, b2,
                     gamma2, beta2, Wout, bout):
    x = np.asarray(x, np.float32)
    seg = np.asarray(seg)
    exp_seg = np.repeat(np.arange(G, dtype=np.int64), _SIZES)
    assert np.array_equal(seg.astype(np.int64), exp_seg), "unexpected seg layout"

    # fold w into x (single bf16 rounding), pad to FP, per-core transpose
    # to the [128, NSUB*BS] streaming layout: xr[p, c*BS+b] = xw[b, 128c+p]
    xw = x * np.asarray(w_flat, np.float32)[None, :]
    xb = np.zeros((B, FP), ml_dtypes.bfloat16)
    xb[:, :F] = xw.astype(ml_dtypes.bfloat16)
    xr = np.ascontiguousarray(
        xb.view(np.uint16).reshape(NCORES, BS, NSUB, 128).transpose(0, 3, 2, 1)
    ).reshape(NCORES, 128, NSUB * BS).view(ml_dtypes.bfloat16)

    ind = (exp_seg[:SUPER_SUBS * 128, None] == np.arange(GBLK)[None, :])
    ind = np.ascontiguousarray(
        ind.astype(ml_dtypes.bfloat16).reshape(SUPER_SUBS, 128, GBLK)
        .transpose(1, 0, 2).reshape(128, SUPER_SUBS * GBLK))
    gbpt = _pack_pt(np.asarray(gene_b, np.float32), GT_TILES)
    w1t_full = np.zeros((GT_TILES * 128, H1), np.float32)
    w1t_full[:G] = np.asarray(W1, np.float32).T
    w1t = np.ascontiguousarray(
        w1t_full.reshape(GT_TILES, 128, H1).transpose(1, 0, 2)
        .reshape(128, GT_TILES * H1)).astype(ml_dtypes.bfloat16)
    w2t = np.ascontiguousarray(
        np.asarray(W2, np.float32).T.reshape(4, 128, H2).transpose(1, 0, 2)
        .reshape(128, 4 * H2)).astype(ml_dtypes.bfloat16)

    ga1 = np.asarray(gamma1, np.float32)
    be1 = np.asarray(beta1, np.float32)
    ga2 = np.asarray(gamma2, np.float32)
    be2 = np.asarray(beta2, np.float32)
    bg1 = np.divide(be1, ga1, out=np.zeros_like(be1), where=(ga1 != 0))
    bg2 = np.divide(be2, ga2, out=np.zeros_like(be2), where=(ga2 != 0))

    bnc = np.zeros((128, 16), np.float32)
    bnc[:, 0:4] = _pack_pt(bg1, 4)
    bnc[:, 4:8] = _pack_pt(ga1, 4)
    bnc[:, 8:10] = _pack_pt(bg2, 2)
    bnc[:, 10:12] = _pack_pt(ga2, 2)
    bnc[:, 12:14] = _pack_pt(np.asarray(Wout, np.float32).reshape(-1), 2)
    bnc[0, 14] = np.asarray(bout, np.float32).reshape(-1)[0]

    consts = dict(ind=ind, gbpt=gbpt, w1t=w1t, w2t=w2t, bnc=bnc)
    return [dict(consts, x=np.ascontiguousarray(xr[i]))
            for i in range(NCORES)]


def _graph():
    global _GRAPH
    if _GRAPH is None:
        _GRAPH = _build_graph()
    return _GRAPH


def _gather(res):
    out = np.concatenate([np.asarray(r["out"]).reshape(-1)
                          for r in res.results])
    return out.reshape(B, 1).astype(np.float32)


def kernel(**inputs):
    in_maps = _prepare_in_maps(**inputs)
    res = run_bass_kernel_spmd(_graph(), in_maps, list(range(NCORES)))
    return _gather(res)
